# revision 12
# baseline (speedup 1.0000x reference)
"""Causal self-attention (B=2, T=2048, C=1024, H=16) on 8 TRN2 NeuronCores.

Sharding: 8 cores = 2 batches x 4 head-groups (4 heads each).
Each core computes qkv for its heads, causal attention, and a partial
output projection; the host sums the 4 partial projections per batch.

All matmuls run in float32r (TF32-like) at 1 cycle/row.

Layouts (per core):
  xT   [C, T]        x^T, streamed in [128, 512] slices
  wT   [C, 768]      qkv weight slice, pre-transposed; column order
                     [q01|k01|v01|q23|k23|v23] so pair-0 weights land first
  qkvT [6][128, T]   m0,m1 = q^T (heads 01, 23); m2,m3 = k^T; m4,m5 = v^T
  vaug [2][128, 2080] per head pair: 16 k-tile groups of 130 cols =
                     [v_h0 (64) | ones | v_h1 (64) | ones]
  attention in scoresT layout: partition = k, free = q. exp on ScalarE
  handles both heads of a pair in one instruction (2-bank PSUM tile).
  av^T accumulated via matmul with v_aug (ones column -> softmax sums).

Emission is interleaved per q-block (qkv chains for block n, then
attention for q-block n) so all engines ramp up early.
"""

import numpy as np

import concourse.bass as bass
import concourse.mybir as mybir
import concourse.tile as tile
from concourse import bacc, bass_utils
F32 = mybir.dt.float32
F32R = mybir.dt.float32r
AF = mybir.ActivationFunctionType

B = 2
T = 2048
C = 1024
D = 64
N_CORES = 8
HG = 4            # heads per core
CG = HG * D       # 256 y-columns per core
P = 128
TQ = 512          # q block width
NKT = T // P      # 16 k tiles
NQB = T // TQ     # 4 q blocks
NCT = C // P      # 8 contraction tiles for qkv
NM = 3 * CG // P  # 6 output m-tiles for qkvT

# logical qkvT m-tile -> physical column slot in wT (pair-0 tiles first)
WCOL = {0: 0, 2: 1, 4: 2, 1: 3, 3: 4, 5: 5}

_cached = {}


def _build_nc():
    nc = bacc.Bacc("TRN2", target_bir_lowering=False, debug=False,
                   num_devices=N_CORES)
    xT = nc.dram_tensor("xT", [C, T], F32, kind="ExternalInput")
    wT = nc.dram_tensor("wT", [C, 3 * CG], F32, kind="ExternalInput")
    pT = nc.dram_tensor("pT", [CG, C], F32, kind="ExternalInput")
    cst = nc.dram_tensor("cst", [P, 576], F32, kind="ExternalInput")
    out = nc.dram_tensor("out", [T, C], F32, kind="ExternalOutput")

    with tile.TileContext(nc) as tc:
        with (
            tc.tile_pool(name="const", bufs=1) as const,
            tc.tile_pool(name="persist", bufs=1) as persist,
            tc.tile_pool(name="xw", bufs=2) as xw,
            tc.tile_pool(name="wpool", bufs=1) as wpool,
            tc.tile_pool(name="esb", bufs=3) as esb,
            tc.tile_pool(name="small", bufs=2) as small,
            tc.tile_pool(name="psSC", bufs=2, space="PSUM") as psSC,
            tc.tile_pool(name="psMM", bufs=2, space="PSUM") as psMM,
            tc.tile_pool(name="psAv", bufs=1, space="PSUM") as psAv,
        ):
            # -------- constants (host-prepared, one DMA) --------
            # layout: [ident 128 | tri2 256 | ones 64 | sel 128]
            cstt = const.tile([P, 576], F32R, tag="cstt")
            nc.sync.dma_start(cstt[:], cst[:].bitcast(F32R))
            ident = cstt[:, 0:P]
            tri2 = cstt[:, P:3 * P]
            onesf = cstt[:, 3 * P:3 * P + D]

            # ---------------- persistent tensors ----------------
            qkvT = [persist.tile([P, T], F32R, tag=f"qkvT{m}", name=f"qkvT{m}")
                    for m in range(NM)]
            vaug = [persist.tile([P, NKT * 2 * (D + 1)], F32R,
                                 tag=f"vaug{j}", name=f"vaug{j}")
                    for j in range(2)]
            yT = [persist.tile([P, T], F32R, tag=f"yT{j}", name=f"yT{j}")
                  for j in range(2)]
            wp = [persist.tile([P, C], F32R, tag=f"wp{j}", name=f"wp{j}")
                  for j in range(2)]

            # ---------------- input DMAs ----------------
            xs = {}
            wt = []
            for n in range(NQB):
                for c in range(NCT):
                    t0 = n * TQ
                    xs[(c, n)] = xw.tile([P, TQ], F32R, tag=f"x{c}",
                                         name=f"x{c}_{n}")
                    nc.sync.dma_start(
                        xs[(c, n)][:],
                        xT[c * P:(c + 1) * P, t0:t0 + TQ].bitcast(F32R))
                if n == 0:
                    for c in range(NCT):
                        w_ = wpool.tile([P, 3 * CG], F32R, tag=f"w{c}",
                                        name=f"w{c}")
                        wt.append(w_)
                        # pair-0 half first, then pair-1 half
                        for h2 in range(2):
                            nc.sync.dma_start(
                                w_[:, h2 * 384:(h2 + 1) * 384],
                                wT[c * P:(c + 1) * P,
                                   h2 * 384:(h2 + 1) * 384].bitcast(F32R))
                    for j in range(2):
                        nc.sync.dma_start(
                            wp[j][:],
                            pT[j * P:(j + 1) * P, :].bitcast(F32R))

            # -------- interleaved: qkv chains + attention per q block --------
            for n in range(NQB):
                qb, q0 = n, n * TQ
                nkt = (n + 1) * (TQ // P)
                for j in range(2):
                    # ---- stage A chains for this (n, pair) ----
                    for m in (j, 2 + j, 4 + j):
                        mc = WCOL[m]
                        ps = psMM.tile([P, TQ], F32, tag="mm",
                                       name=f"psA_{n}_{m}")
                        for c in range(NCT):
                            nc.tensor.matmul(
                                ps[:],
                                wt[c][:, mc * P:(mc + 1) * P],
                                xs[(c, n)][:],
                                start=(c == 0), stop=(c == NCT - 1),
                            )
                        nc.vector.tensor_copy(
                            qkvT[m][:, n * TQ:(n + 1) * TQ], ps[:])

                    # ---- stage B: v transposes for new k tiles ----
                    vr = vaug[j].rearrange("p (k g x) -> p k g x", k=NKT, g=2)
                    for kt in range(4 * n, 4 * n + 4):
                        pt = psMM.tile([P, P], F32R, tag="mm",
                                       name=f"pt_{j}_{kt}")
                        nc.tensor.transpose(
                            pt[:], qkvT[4 + j][:, kt * P:(kt + 1) * P],
                            ident)
                        nc.vector.tensor_copy(
                            vr[:, kt, :, :D],
                            pt[:].rearrange("p (g x) -> p g x", g=2))
                    nc.vector.tensor_copy(
                        vaug[j][:, D + 65 * 8 * n:(D + 1) * 2 * (4 * n + 4):D + 1],
                        onesf[:, :8])

                    # ---- stage C: attention for (qb, pair j) ----
                    qm, km = qkvT[j], qkvT[2 + j]
                    avp = [psAv.tile([D + 1, TQ], F32, tag=f"av{hh}",
                                     name=f"av{hh}_{qb}_{j}")
                           for hh in range(2)]
                    for kt in range(nkt):
                        z = max(0, kt * P - q0)
                        sc = psSC.tile([P, 2 * TQ], F32, tag="sc",
                                       name=f"sc_{qb}_{j}_{kt}")
                        scr = sc.rearrange("p (g x) -> p g x", g=2)
                        ee = esb.tile([P, 2 * TQ], F32R, tag="ee",
                                      name=f"ee_{qb}_{j}_{kt}")
                        eer = ee.rearrange("p (g x) -> p g x", g=2)
                        for hh in range(2):
                            nc.tensor.matmul(
                                scr[:, hh, z:],
                                km[hh * D:(hh + 1) * D, kt * P:(kt + 1) * P],
                                qm[hh * D:(hh + 1) * D, q0 + z:q0 + TQ],
                                start=True, stop=True,
                                tile_position=(hh * D, 0),
                            )
                        nc.scalar.activation(
                            eer[:, :, z:], scr[:, :, z:], AF.Exp, scale=0.125)
                        if kt * P >= q0:  # diagonal band: triangular mask
                            nc.vector.tensor_mul(
                                eer[:, :, z:z + P], eer[:, :, z:z + P],
                                tri2.rearrange("p (g x) -> p g x", g=2))
                        for hh in range(2):
                            nc.tensor.matmul(
                                avp[hh][:, z:],
                                vr[:, kt, hh, :],
                                eer[:, hh, z:],
                                start=(kt == 0), stop=(kt == nkt - 1),
                            )
                    # softmax division: yT = avT * (1/sums) broadcast
                    for hh in range(2):
                        rp = small.tile([1, TQ], F32R, tag="rp",
                                        name=f"rp_{qb}_{j}_{hh}")
                        with nc.allow_low_precision(reason="f32r recip"):
                            nc.vector.reciprocal(rp[:], avp[hh][D:D + 1, :])
                        bc = psMM.tile([D, TQ], F32, tag="mm",
                                       name=f"bc_{qb}_{j}_{hh}")
                        nc.tensor.matmul(bc[:], onesf[0:1, :], rp[:],
                                         start=True, stop=True)
                        bcs = small.tile([D, TQ], F32R, tag="bcs",
                                         name=f"bcs_{qb}_{j}_{hh}")
                        nc.vector.tensor_copy(bcs[:], bc[:])
                        nc.vector.tensor_mul(
                            yT[j][hh * D:(hh + 1) * D, q0:q0 + TQ],
                            avp[hh][:D, :], bcs[:])

                # ---- stage D: proj for this q block ----
                for tb in range(qb * (TQ // P), (qb + 1) * (TQ // P)):
                    for oh in range(2):
                        pp = psMM.tile([P, TQ], F32, tag="mm",
                                       name=f"pp_{tb}_{oh}")
                        for cc in range(2):
                            nc.tensor.matmul(
                                pp[:],
                                yT[cc][:, tb * P:(tb + 1) * P],
                                wp[cc][:, oh * TQ:(oh + 1) * TQ],
                                start=(cc == 0), stop=(cc == 1),
                            )
                        ob = esb.tile([P, TQ], F32, tag="ob",
                                      name=f"ob_{tb}_{oh}")
                        if oh == 0:
                            nc.vector.tensor_copy(ob[:], pp[:])
                        else:
                            nc.scalar.copy(ob[:], pp[:])
                        nc.sync.dma_start(
                            out[tb * P:(tb + 1) * P, oh * TQ:(oh + 1) * TQ],
                            ob[:])

    nc.compile()
    return nc


def _prep_inputs(x, w_qkv, w_proj):
    """Build per-core input maps. Core c = b * 4 + hg."""
    in_maps = []
    xTb = [np.ascontiguousarray(x[b].T) for b in range(B)]
    cst = np.zeros((P, 576), dtype=np.float32)
    cst[:, 0:P] = np.eye(P, dtype=np.float32)
    tri = np.triu(np.ones((P, P), dtype=np.float32))  # 1 where pk <= fq
    cst[:, P:2 * P] = tri
    cst[:, 2 * P:3 * P] = tri
    cst[:, 3 * P:3 * P + D] = 1.0
    cst[0, 3 * P + D:3 * P + D + D] = 1.0        # sel row 0: cols 0:64
    cst[1, 3 * P + 2 * D:3 * P + 3 * D] = 1.0    # sel row 1: cols 64:128
    for b in range(B):
        for hg in range(HG):
            sl = slice(hg * CG, (hg + 1) * CG)
            q, k, v = w_qkv[sl], w_qkv[C:][sl], w_qkv[2 * C:][sl]
            # physical column order: q01 k01 v01 | q23 k23 v23
            wTg = np.ascontiguousarray(np.concatenate(
                [q[:P], k[:P], v[:P], q[P:], k[P:], v[P:]], axis=0).T)
            pTg = np.ascontiguousarray(w_proj[:, sl].T)
            in_maps.append({"xT": xTb[b], "wT": wTg, "pT": pTg, "cst": cst})
    return in_maps


def kernel(x, w_qkv, w_proj):
    x = np.asarray(x, dtype=np.float32)
    w_qkv = np.asarray(w_qkv, dtype=np.float32)
    w_proj = np.asarray(w_proj, dtype=np.float32)

    if "nc" not in _cached:
        _cached["nc"] = _build_nc()
    nc = _cached["nc"]

    in_maps = _prep_inputs(x, w_qkv, w_proj)
    res = bass_utils.run_bass_kernel_spmd(nc, in_maps, core_ids=list(range(N_CORES)))

    out = np.zeros((B, T, C), dtype=np.float32)
    for b in range(B):
        for hg in range(HG):
            out[b] += res.results[b * HG + hg]["out"]
    return out


# revision 13
# speedup vs baseline: 1.2083x; 1.2083x over previous
"""Causal self-attention (B=2, T=2048, C=1024, H=16) on 8 TRN2 NeuronCores.

Sharding: 8 cores = 2 batches x 4 head-groups (4 heads each).
Each core computes qkv for its heads, causal attention, and a partial
output projection; the host sums the 4 partial projections per batch.

All matmuls run in float32r (TF32-like) at 1 cycle/row.

Layouts (per core):
  xT   [C, T]        x^T, streamed in [128, 512] slices
  wT   [C, 768]      qkv weight slice, pre-transposed; column order
                     [q01|k01|v01|q23|k23|v23] so pair-0 weights land first
  qkvT [6][128, T]   m0,m1 = q^T (heads 01, 23); m2,m3 = k^T; m4,m5 = v^T
  vaug [2][128, 2080] per head pair: 16 k-tile groups of 130 cols =
                     [v_h0 (64) | ones | v_h1 (64) | ones]
  attention in scoresT layout: partition = k, free = q. exp on ScalarE
  handles both heads of a pair in one instruction (2-bank PSUM tile).
  av^T accumulated via matmul with v_aug (ones column -> softmax sums).

Emission is interleaved per q-block (qkv chains for block n, then
attention for q-block n) so all engines ramp up early.
"""

import numpy as np

import concourse.bass as bass
import concourse.mybir as mybir
import concourse.tile as tile
from concourse import bacc, bass_utils
F32 = mybir.dt.float32
F32R = mybir.dt.float32r
AF = mybir.ActivationFunctionType

B = 2
T = 2048
C = 1024
D = 64
N_CORES = 8
HG = 4            # heads per core
CG = HG * D       # 256 y-columns per core
P = 128
TQ = 512          # q block width
NKT = T // P      # 16 k tiles
NQB = T // TQ     # 4 q blocks
NCT = C // P      # 8 contraction tiles for qkv
NM = 3 * CG // P  # 6 output m-tiles for qkvT

# logical qkvT m-tile -> physical column slot in wT (pair-0 tiles first)
WCOL = {0: 0, 2: 1, 4: 2, 1: 3, 3: 4, 5: 5}

_cached = {}


def _build_nc():
    nc = bacc.Bacc("TRN2", target_bir_lowering=False, debug=False,
                   num_devices=N_CORES)
    xT = nc.dram_tensor("xT", [C, T], F32, kind="ExternalInput")
    wT = nc.dram_tensor("wT", [C, 3 * CG], F32, kind="ExternalInput")
    pT = nc.dram_tensor("pT", [CG, C], F32, kind="ExternalInput")
    cst = nc.dram_tensor("cst", [P, 576], F32, kind="ExternalInput")
    out = nc.dram_tensor("out", [T, C], F32, kind="ExternalOutput")

    with tile.TileContext(nc) as tc:
        with (
            tc.tile_pool(name="const", bufs=1) as const,
            tc.tile_pool(name="persist", bufs=1) as persist,
            tc.tile_pool(name="xw", bufs=2) as xw,
            tc.tile_pool(name="wpool", bufs=1) as wpool,
            tc.tile_pool(name="esb", bufs=3) as esb,
            tc.tile_pool(name="small", bufs=2) as small,
            tc.tile_pool(name="psSC", bufs=2, space="PSUM") as psSC,
            tc.tile_pool(name="psMM", bufs=2, space="PSUM") as psMM,
            tc.tile_pool(name="psAv", bufs=1, space="PSUM") as psAv,
        ):
            # -------- constants (host-prepared, one DMA) --------
            # layout: [ident 128 | tri2 256 | ones 64 | sel 128]
            cstt = const.tile([P, 576], F32R, tag="cstt")
            nc.sync.dma_start(cstt[:], cst[:].bitcast(F32R))
            ident = cstt[:, 0:P]
            tri2 = cstt[:, P:3 * P]
            onesf = cstt[:, 3 * P:3 * P + D]

            # ---------------- persistent tensors ----------------
            qkvT = [persist.tile([P, T], F32R, tag=f"qkvT{m}", name=f"qkvT{m}")
                    for m in range(NM)]
            vaug = [persist.tile([P, NKT * 2 * (D + 1)], F32R,
                                 tag=f"vaug{j}", name=f"vaug{j}")
                    for j in range(2)]
            yT = [persist.tile([P, T], F32R, tag=f"yT{j}", name=f"yT{j}")
                  for j in range(2)]
            wp = [persist.tile([P, C], F32R, tag=f"wp{j}", name=f"wp{j}")
                  for j in range(2)]

            # ---------------- input DMAs ----------------
            xs = {}
            wt = []
            for n in range(NQB):
                for c in range(NCT):
                    t0 = n * TQ
                    xs[(c, n)] = xw.tile([P, TQ], F32R, tag=f"x{c}",
                                         name=f"x{c}_{n}")
                    nc.sync.dma_start(
                        xs[(c, n)][:],
                        xT[c * P:(c + 1) * P, t0:t0 + TQ].bitcast(F32R))
                if n == 0:
                    for c in range(NCT):
                        w_ = wpool.tile([P, 3 * CG], F32R, tag=f"w{c}",
                                        name=f"w{c}")
                        wt.append(w_)
                        # pair-0 half first, then pair-1 half
                        for h2 in range(2):
                            nc.sync.dma_start(
                                w_[:, h2 * 384:(h2 + 1) * 384],
                                wT[c * P:(c + 1) * P,
                                   h2 * 384:(h2 + 1) * 384].bitcast(F32R))
                    for j in range(2):
                        nc.sync.dma_start(
                            wp[j][:],
                            pT[j * P:(j + 1) * P, :].bitcast(F32R))

            # -------- software-pipelined emission --------
            # PE executes its stream in order, so QKV chains / v-transposes
            # for block n+1 and proj matmuls for block qb-1 are interleaved
            # into the (ACT-bound) attention k-loop of block qb.
            vrs = [vaug[j].rearrange("p (k g x) -> p k g x", k=NKT, g=2)
                   for j in range(2)]

            def emit_chain(n, m):
                mc = WCOL[m]
                ps = psMM.tile([P, TQ], F32, tag="mm", name=f"psA_{n}_{m}")
                for c in range(NCT):
                    nc.tensor.matmul(
                        ps[:],
                        wt[c][:, mc * P:(mc + 1) * P],
                        xs[(c, n)][:],
                        start=(c == 0), stop=(c == NCT - 1),
                    )
                nc.vector.tensor_copy(qkvT[m][:, n * TQ:(n + 1) * TQ], ps[:])

            def emit_pt(j, kt):
                pt = psMM.tile([P, P], F32R, tag="mm", name=f"pt_{j}_{kt}")
                nc.tensor.transpose(
                    pt[:], qkvT[4 + j][:, kt * P:(kt + 1) * P], ident)
                nc.vector.tensor_copy(
                    vrs[j][:, kt, :, :D],
                    pt[:].rearrange("p (g x) -> p g x", g=2))

            def emit_ones(n, j):
                nc.vector.tensor_copy(
                    vaug[j][:, D + 65 * 8 * n:130 * (4 * n + 4):D + 1],
                    onesf[:, :8])

            def emit_pp(tb, oh, engine):
                pp = psMM.tile([P, TQ], F32, tag="mm", name=f"pp_{tb}_{oh}")
                for cc in range(2):
                    nc.tensor.matmul(
                        pp[:],
                        yT[cc][:, tb * P:(tb + 1) * P],
                        wp[cc][:, oh * TQ:(oh + 1) * TQ],
                        start=(cc == 0), stop=(cc == 1),
                    )
                ob = esb.tile([P, TQ], F32, tag="ob", name=f"ob_{tb}_{oh}")
                if engine == "act":
                    nc.scalar.copy(ob[:], pp[:])
                else:
                    nc.vector.tensor_copy(ob[:], pp[:])
                nc.sync.dma_start(
                    out[tb * P:(tb + 1) * P, oh * TQ:(oh + 1) * TQ], ob[:])

            pending = []  # (segment_stashed, callable)

            def drain(upto_seg):
                while pending and pending[0][0] <= upto_seg:
                    pending.pop(0)[1]()

            # prologue: block 0 qkv + v-transposes for both pairs
            for j in range(2):
                for m in (j, 2 + j, 4 + j):
                    emit_chain(0, m)
                for kt in range(4):
                    emit_pt(j, kt)
                emit_ones(0, j)

            for qb in range(NQB):
                q0 = qb * TQ
                nkt = (qb + 1) * (TQ // P)
                for j in range(2):
                    s = 2 * qb + j
                    drain(s - 2)
                    # stash next-block qkv work for this pair
                    if qb + 1 < NQB:
                        n = qb + 1
                        for m in (j, 2 + j, 4 + j):
                            pending.append((s, (lambda n=n, m=m: emit_chain(n, m))))
                        for kt in range(4 * n, 4 * n + 4):
                            pending.append((s, (lambda j=j, kt=kt: emit_pt(j, kt))))
                        pending.append((s, (lambda n=n, j=j: emit_ones(n, j))))
                    # stash proj for the previous q block
                    if j == 0 and qb >= 1:
                        for tb in range((qb - 1) * 4, qb * 4):
                            for oh in range(2):
                                pending.append(
                                    (s, (lambda tb=tb, oh=oh: emit_pp(tb, oh, "dve"))))

                    # ---- attention k-loop for (qb, pair j) ----
                    qm, km = qkvT[j], qkvT[2 + j]
                    avp = [psAv.tile([D + 1, TQ], F32, tag=f"av{hh}",
                                     name=f"av{hh}_{qb}_{j}")
                           for hh in range(2)]
                    for kt in range(nkt):
                        z = max(0, kt * P - q0)
                        sc = psSC.tile([P, 2 * TQ], F32, tag="sc",
                                       name=f"sc_{qb}_{j}_{kt}")
                        scr = sc.rearrange("p (g x) -> p g x", g=2)
                        ee = esb.tile([P, 2 * TQ], F32R, tag="ee",
                                      name=f"ee_{qb}_{j}_{kt}")
                        eer = ee.rearrange("p (g x) -> p g x", g=2)
                        for hh in range(2):
                            nc.tensor.matmul(
                                scr[:, hh, z:],
                                km[hh * D:(hh + 1) * D, kt * P:(kt + 1) * P],
                                qm[hh * D:(hh + 1) * D, q0 + z:q0 + TQ],
                                start=True, stop=True,
                                tile_position=(hh * D, 0),
                            )
                        nc.scalar.activation(
                            eer[:, :, z:], scr[:, :, z:], AF.Exp, scale=0.125)
                        if kt * P >= q0:  # diagonal band: triangular mask
                            nc.vector.tensor_mul(
                                eer[:, :, z:z + P], eer[:, :, z:z + P],
                                tri2.rearrange("p (g x) -> p g x", g=2))
                        for hh in range(2):
                            nc.tensor.matmul(
                                avp[hh][:, z:],
                                vrs[j][:, kt, hh, :],
                                eer[:, hh, z:],
                                start=(kt == 0), stop=(kt == nkt - 1),
                            )
                        # pace deferred PE work into the ACT-bound loop
                        iters_left = nkt - kt
                        k = -(-len(pending) // iters_left)  # ceil
                        for _ in range(min(k, len(pending))):
                            pending.pop(0)[1]()

                    # softmax division: yT = avT * (1/sums) broadcast
                    for hh in range(2):
                        rp = small.tile([1, TQ], F32R, tag="rp",
                                        name=f"rp_{qb}_{j}_{hh}")
                        with nc.allow_low_precision(reason="f32r recip"):
                            nc.vector.reciprocal(rp[:], avp[hh][D:D + 1, :])
                        bc = psMM.tile([D, TQ], F32, tag="mm",
                                       name=f"bc_{qb}_{j}_{hh}")
                        nc.tensor.matmul(bc[:], onesf[0:1, :], rp[:],
                                         start=True, stop=True)
                        bcs = small.tile([D, TQ], F32R, tag="bcs",
                                         name=f"bcs_{qb}_{j}_{hh}")
                        nc.vector.tensor_copy(bcs[:], bc[:])
                        nc.vector.tensor_mul(
                            yT[j][hh * D:(hh + 1) * D, q0:q0 + TQ],
                            avp[hh][:D, :], bcs[:])

            # tail: proj for the last q block (split copies DVE/ACT)
            for tb in range(3 * 4, 4 * 4):
                for oh in range(2):
                    emit_pp(tb, oh, "act" if (tb + oh) % 2 else "dve")

    nc.compile()
    return nc


def _prep_inputs(x, w_qkv, w_proj):
    """Build per-core input maps. Core c = b * 4 + hg."""
    in_maps = []
    xTb = [np.ascontiguousarray(x[b].T) for b in range(B)]
    cst = np.zeros((P, 576), dtype=np.float32)
    cst[:, 0:P] = np.eye(P, dtype=np.float32)
    tri = np.triu(np.ones((P, P), dtype=np.float32))  # 1 where pk <= fq
    cst[:, P:2 * P] = tri
    cst[:, 2 * P:3 * P] = tri
    cst[:, 3 * P:3 * P + D] = 1.0
    cst[0, 3 * P + D:3 * P + D + D] = 1.0        # sel row 0: cols 0:64
    cst[1, 3 * P + 2 * D:3 * P + 3 * D] = 1.0    # sel row 1: cols 64:128
    for b in range(B):
        for hg in range(HG):
            sl = slice(hg * CG, (hg + 1) * CG)
            q, k, v = w_qkv[sl], w_qkv[C:][sl], w_qkv[2 * C:][sl]
            # physical column order: q01 k01 v01 | q23 k23 v23
            wTg = np.ascontiguousarray(np.concatenate(
                [q[:P], k[:P], v[:P], q[P:], k[P:], v[P:]], axis=0).T)
            pTg = np.ascontiguousarray(w_proj[:, sl].T)
            in_maps.append({"xT": xTb[b], "wT": wTg, "pT": pTg, "cst": cst})
    return in_maps


def kernel(x, w_qkv, w_proj):
    x = np.asarray(x, dtype=np.float32)
    w_qkv = np.asarray(w_qkv, dtype=np.float32)
    w_proj = np.asarray(w_proj, dtype=np.float32)

    if "nc" not in _cached:
        _cached["nc"] = _build_nc()
    nc = _cached["nc"]

    in_maps = _prep_inputs(x, w_qkv, w_proj)
    res = bass_utils.run_bass_kernel_spmd(nc, in_maps, core_ids=list(range(N_CORES)))

    out = np.zeros((B, T, C), dtype=np.float32)
    for b in range(B):
        for hg in range(HG):
            out[b] += res.results[b * HG + hg]["out"]
    return out


# revision 14
# speedup vs baseline: 1.2328x; 1.0203x over previous
"""Causal self-attention (B=2, T=2048, C=1024, H=16) on 8 TRN2 NeuronCores.

Sharding: 8 cores = 2 batches x 4 head-groups (4 heads each).
Each core computes qkv for its heads, causal attention, and a partial
output projection; the host sums the 4 partial projections per batch.

All matmuls run in float32r (TF32-like) at 1 cycle/row.

Layouts (per core):
  xT   [C, T]        x^T, streamed in [128, 512] slices
  wT   [C, 768]      qkv weight slice, pre-transposed; column order
                     [q01|k01|v01|q23|k23|v23] so pair-0 weights land first
  qkvT [6][128, T]   m0,m1 = q^T (heads 01, 23); m2,m3 = k^T; m4,m5 = v^T
  vaug [2][128, 2080] per head pair: 16 k-tile groups of 130 cols =
                     [v_h0 (64) | ones | v_h1 (64) | ones]
  attention in scoresT layout: partition = k, free = q. exp on ScalarE
  handles both heads of a pair in one instruction (2-bank PSUM tile).
  av^T accumulated via matmul with v_aug (ones column -> softmax sums).

Emission is interleaved per q-block (qkv chains for block n, then
attention for q-block n) so all engines ramp up early.
"""

import numpy as np

import concourse.bass as bass
import concourse.mybir as mybir
import concourse.tile as tile
from concourse import bacc, bass_utils
F32 = mybir.dt.float32
F32R = mybir.dt.float32r
AF = mybir.ActivationFunctionType

B = 2
T = 2048
C = 1024
D = 64
N_CORES = 8
HG = 4            # heads per core
CG = HG * D       # 256 y-columns per core
P = 128
TQ = 512          # q block width
NKT = T // P      # 16 k tiles
NQB = T // TQ     # 4 q blocks
NCT = C // P      # 8 contraction tiles for qkv
NM = 3 * CG // P  # 6 output m-tiles for qkvT

# logical qkvT m-tile -> physical column slot in wT (pair-0 tiles first)
WCOL = {0: 0, 2: 1, 4: 2, 1: 3, 3: 4, 5: 5}

_cached = {}


def _build_nc():
    nc = bacc.Bacc("TRN2", target_bir_lowering=False, debug=False,
                   num_devices=N_CORES)
    xT = nc.dram_tensor("xT", [C, T], F32, kind="ExternalInput")
    wT = nc.dram_tensor("wT", [C, 3 * CG], F32, kind="ExternalInput")
    pT = nc.dram_tensor("pT", [CG, C], F32, kind="ExternalInput")
    cst = nc.dram_tensor("cst", [P, 576], F32, kind="ExternalInput")
    out = nc.dram_tensor("out", [T, C], F32, kind="ExternalOutput")

    with tile.TileContext(nc) as tc:
        with (
            tc.tile_pool(name="const", bufs=1) as const,
            tc.tile_pool(name="persist", bufs=1) as persist,
            tc.tile_pool(name="xw", bufs=2) as xw,
            tc.tile_pool(name="wpool", bufs=1) as wpool,
            tc.tile_pool(name="esb", bufs=3) as esb,
            tc.tile_pool(name="small", bufs=2) as small,
            tc.tile_pool(name="psSC", bufs=2, space="PSUM") as psSC,
            tc.tile_pool(name="psMM", bufs=2, space="PSUM") as psMM,
            tc.tile_pool(name="psAv", bufs=1, space="PSUM") as psAv,
        ):
            # -------- constants (host-prepared, one DMA) --------
            # layout: [ident 128 | tri2 256 | ones 64 | sel 128]
            cstt = const.tile([P, 576], F32R, tag="cstt")
            nc.sync.dma_start(cstt[:], cst[:].bitcast(F32R))
            ident = cstt[:, 0:P]
            tri2 = cstt[:, P:3 * P]
            onesf = cstt[:, 3 * P:3 * P + D]

            # ---------------- persistent tensors ----------------
            qkvT = [persist.tile([P, T], F32R, tag=f"qkvT{m}", name=f"qkvT{m}")
                    for m in range(NM)]
            vaug = [persist.tile([P, NKT * 2 * (D + 1)], F32R,
                                 tag=f"vaug{j}", name=f"vaug{j}")
                    for j in range(2)]
            yT = [persist.tile([P, T], F32R, tag=f"yT{j}", name=f"yT{j}")
                  for j in range(2)]
            wp = [persist.tile([P, C], F32R, tag=f"wp{j}", name=f"wp{j}")
                  for j in range(2)]

            # ---------------- input DMAs ----------------
            xs = {}
            wt = []
            for n in range(NQB):
                for c in range(NCT):
                    t0 = n * TQ
                    xs[(c, n)] = xw.tile([P, TQ], F32R, tag=f"x{c}",
                                         name=f"x{c}_{n}")
                    if n == 0:
                        # interleave x and W per c so chain c-steps start early
                        w_ = wpool.tile([P, 3 * CG], F32R, tag=f"w{c}",
                                        name=f"w{c}")
                        wt.append(w_)
                        nc.sync.dma_start(
                            w_[:, :384],
                            wT[c * P:(c + 1) * P, :384].bitcast(F32R))
                        nc.sync.dma_start(
                            xs[(c, n)][:],
                            xT[c * P:(c + 1) * P, t0:t0 + TQ].bitcast(F32R))
                        nc.sync.dma_start(
                            w_[:, 384:],
                            wT[c * P:(c + 1) * P, 384:].bitcast(F32R))
                    else:
                        nc.sync.dma_start(
                            xs[(c, n)][:],
                            xT[c * P:(c + 1) * P, t0:t0 + TQ].bitcast(F32R))
                if n == 0:
                    for j in range(2):
                        nc.sync.dma_start(
                            wp[j][:],
                            pT[j * P:(j + 1) * P, :].bitcast(F32R))

            # -------- software-pipelined emission --------
            # PE executes its stream in order, so QKV chains / v-transposes
            # for block n+1 and proj matmuls for block qb-1 are interleaved
            # into the (ACT-bound) attention k-loop of block qb.
            vrs = [vaug[j].rearrange("p (k g x) -> p k g x", k=NKT, g=2)
                   for j in range(2)]

            def emit_chain(n, m):
                mc = WCOL[m]
                ps = psMM.tile([P, TQ], F32, tag="mm", name=f"psA_{n}_{m}")
                for c in range(NCT):
                    nc.tensor.matmul(
                        ps[:],
                        wt[c][:, mc * P:(mc + 1) * P],
                        xs[(c, n)][:],
                        start=(c == 0), stop=(c == NCT - 1),
                    )
                nc.vector.tensor_copy(qkvT[m][:, n * TQ:(n + 1) * TQ], ps[:])

            def emit_pt(j, kt):
                pt = psMM.tile([P, P], F32R, tag="mm", name=f"pt_{j}_{kt}")
                nc.tensor.transpose(
                    pt[:], qkvT[4 + j][:, kt * P:(kt + 1) * P], ident)
                nc.vector.tensor_copy(
                    vrs[j][:, kt, :, :D],
                    pt[:].rearrange("p (g x) -> p g x", g=2))

            def emit_ones(n, j):
                nc.vector.tensor_copy(
                    vaug[j][:, D + 65 * 8 * n:130 * (4 * n + 4):D + 1],
                    onesf[:, :8])

            def emit_pp(tb, oh, engine):
                pp = psMM.tile([P, TQ], F32, tag="mm", name=f"pp_{tb}_{oh}")
                for cc in range(2):
                    nc.tensor.matmul(
                        pp[:],
                        yT[cc][:, tb * P:(tb + 1) * P],
                        wp[cc][:, oh * TQ:(oh + 1) * TQ],
                        start=(cc == 0), stop=(cc == 1),
                    )
                ob = esb.tile([P, TQ], F32, tag="ob", name=f"ob_{tb}_{oh}")
                if engine == "act":
                    nc.scalar.copy(ob[:], pp[:])
                else:
                    nc.vector.tensor_copy(ob[:], pp[:])
                nc.sync.dma_start(
                    out[tb * P:(tb + 1) * P, oh * TQ:(oh + 1) * TQ], ob[:])

            pending = []  # (segment_stashed, callable)

            def drain(upto_seg):
                while pending and pending[0][0] <= upto_seg:
                    pending.pop(0)[1]()

            # prologue: block 0 qkv + v-transposes for both pairs
            for j in range(2):
                for m in (j, 2 + j, 4 + j):
                    emit_chain(0, m)
                for kt in range(4):
                    emit_pt(j, kt)
                emit_ones(0, j)

            for qb in range(NQB):
                q0 = qb * TQ
                nkt = (qb + 1) * (TQ // P)
                for j in range(2):
                    s = 2 * qb + j
                    drain(s - 2)
                    # stash next-block qkv work for this pair
                    if qb + 1 < NQB:
                        n = qb + 1
                        for m in (j, 2 + j, 4 + j):
                            pending.append((s, (lambda n=n, m=m: emit_chain(n, m))))
                        for kt in range(4 * n, 4 * n + 4):
                            pending.append((s, (lambda j=j, kt=kt: emit_pt(j, kt))))
                        pending.append((s, (lambda n=n, j=j: emit_ones(n, j))))
                    # stash proj for the previous q block (half per pair)
                    if qb >= 1:
                        for tb in range((qb - 1) * 4 + 2 * j,
                                        (qb - 1) * 4 + 2 * j + 2):
                            for oh in range(2):
                                pending.append(
                                    (s, (lambda tb=tb, oh=oh: emit_pp(tb, oh, "dve"))))

                    # ---- attention k-loop for (qb, pair j) ----
                    qm, km = qkvT[j], qkvT[2 + j]
                    avp = [psAv.tile([D + 1, TQ], F32, tag=f"av{hh}",
                                     name=f"av{hh}_{qb}_{j}")
                           for hh in range(2)]
                    for kt in range(nkt):
                        z = max(0, kt * P - q0)
                        sc = psSC.tile([P, 2 * TQ], F32, tag="sc",
                                       name=f"sc_{qb}_{j}_{kt}")
                        scr = sc.rearrange("p (g x) -> p g x", g=2)
                        ee = esb.tile([P, 2 * TQ], F32R, tag="ee",
                                      name=f"ee_{qb}_{j}_{kt}")
                        eer = ee.rearrange("p (g x) -> p g x", g=2)
                        for hh in range(2):
                            nc.tensor.matmul(
                                scr[:, hh, z:],
                                km[hh * D:(hh + 1) * D, kt * P:(kt + 1) * P],
                                qm[hh * D:(hh + 1) * D, q0 + z:q0 + TQ],
                                start=True, stop=True,
                                tile_position=(hh * D, 0),
                            )
                        nc.scalar.activation(
                            eer[:, :, z:], scr[:, :, z:], AF.Exp, scale=0.125)
                        if kt * P >= q0:  # diagonal band: triangular mask
                            nc.vector.tensor_mul(
                                eer[:, :, z:z + P], eer[:, :, z:z + P],
                                tri2.rearrange("p (g x) -> p g x", g=2))
                        for hh in range(2):
                            nc.tensor.matmul(
                                avp[hh][:, z:],
                                vrs[j][:, kt, hh, :],
                                eer[:, hh, z:],
                                start=(kt == 0), stop=(kt == nkt - 1),
                            )
                        # pace deferred PE work into the ACT-bound loop
                        iters_left = nkt - kt
                        k = 1 if len(pending) <= iters_left else 2
                        for _ in range(min(k, len(pending))):
                            pending.pop(0)[1]()

                    # softmax division: yT = avT * (1/sums) broadcast
                    for hh in range(2):
                        rp = small.tile([1, TQ], F32R, tag="rp",
                                        name=f"rp_{qb}_{j}_{hh}")
                        with nc.allow_low_precision(reason="f32r recip"):
                            nc.vector.reciprocal(rp[:], avp[hh][D:D + 1, :])
                        bc = psMM.tile([D, TQ], F32, tag="mm",
                                       name=f"bc_{qb}_{j}_{hh}")
                        nc.tensor.matmul(bc[:], onesf[0:1, :], rp[:],
                                         start=True, stop=True)
                        bcs = small.tile([D, TQ], F32R, tag="bcs",
                                         name=f"bcs_{qb}_{j}_{hh}")
                        nc.vector.tensor_copy(bcs[:], bc[:])
                        nc.vector.tensor_mul(
                            yT[j][hh * D:(hh + 1) * D, q0:q0 + TQ],
                            avp[hh][:D, :], bcs[:])

            # tail: proj for the last q block (split copies DVE/ACT)
            for tb in range(3 * 4, 4 * 4):
                for oh in range(2):
                    emit_pp(tb, oh, "act" if (tb + oh) % 2 else "dve")

    nc.compile()
    return nc


def _prep_inputs(x, w_qkv, w_proj):
    """Build per-core input maps. Core c = b * 4 + hg."""
    in_maps = []
    xTb = [np.ascontiguousarray(x[b].T) for b in range(B)]
    cst = np.zeros((P, 576), dtype=np.float32)
    cst[:, 0:P] = np.eye(P, dtype=np.float32)
    tri = np.triu(np.ones((P, P), dtype=np.float32))  # 1 where pk <= fq
    cst[:, P:2 * P] = tri
    cst[:, 2 * P:3 * P] = tri
    cst[:, 3 * P:3 * P + D] = 1.0
    cst[0, 3 * P + D:3 * P + D + D] = 1.0        # sel row 0: cols 0:64
    cst[1, 3 * P + 2 * D:3 * P + 3 * D] = 1.0    # sel row 1: cols 64:128
    for b in range(B):
        for hg in range(HG):
            sl = slice(hg * CG, (hg + 1) * CG)
            q, k, v = w_qkv[sl], w_qkv[C:][sl], w_qkv[2 * C:][sl]
            # physical column order: q01 k01 v01 | q23 k23 v23
            wTg = np.ascontiguousarray(np.concatenate(
                [q[:P], k[:P], v[:P], q[P:], k[P:], v[P:]], axis=0).T)
            pTg = np.ascontiguousarray(w_proj[:, sl].T)
            in_maps.append({"xT": xTb[b], "wT": wTg, "pT": pTg, "cst": cst})
    return in_maps


def kernel(x, w_qkv, w_proj):
    x = np.asarray(x, dtype=np.float32)
    w_qkv = np.asarray(w_qkv, dtype=np.float32)
    w_proj = np.asarray(w_proj, dtype=np.float32)

    if "nc" not in _cached:
        _cached["nc"] = _build_nc()
    nc = _cached["nc"]

    in_maps = _prep_inputs(x, w_qkv, w_proj)
    res = bass_utils.run_bass_kernel_spmd(nc, in_maps, core_ids=list(range(N_CORES)))

    out = np.zeros((B, T, C), dtype=np.float32)
    for b in range(B):
        for hg in range(HG):
            out[b] += res.results[b * HG + hg]["out"]
    return out
